# revision 69
# baseline (speedup 1.0000x reference)
"""Trainium2 Bass kernel for nn_MultiHeadedSelfAttention_5179730559275.

Reference math (per batch b):
  q = wq @ x + bq ; k = wk @ x + bk ; v = wv @ x + bv        (1x1 conv, C=256 -> O=256)
  per o-channel (o = head*32 + d), with Q_o,K_o,V_o = 64x64 images [H,W]:
    S_o = Q_o @ K_o^T / sqrt(32); P_o = softmax(S_o, axis=-1); ctx_o = P_o @ V_o

Sharding: data-parallel over batch, 2 batches per core on 8 cores.

Per-core pipeline (per batch):
  1. fp16 projections on PE -> psum [o', 512]; ACT/DVE copies add bias +
     cast fp16 into pair-interleaved 8-row tiles [j, r, om, c] (j = o mod
     128, om pairs o with o+128). V projects from host-transposed x so its
     rows are w-major.
  2. PE transposes (is_transpose matmul vs fp16 identity) flip 128x128
     slices [j, (om,c)] -> [(om,c), j], 8 per psum bank; one batched copy
     per bank builds j-major layouts (contiguous per-j tiles so PE weight
     loads / moving fetches run at 1 elem/cycle):
       qS/kS: [om*64 + w, j, h] ; vS: [om*64 + g, j, w] (+ ones col for Z)
     Transposes for row-group nt interleave with the projections of nt+1
     so the PE never waits on the psT drain copies.
  3. Attention per pair j: quadrant matmuls (K=64 at partition bases 0/64):
       S^T psum [om*64+g, h] ; exp (ACT, bias -2) -> eS fp16
       ctx psum [om*64+h, 0:64]=E^T.T@V, col 64 = Z (ones column)
     normalize (bv folded into V bias): obuf = psum * (1/Z); one 16KB-
     descriptor DMA store per om per batch (DRAM [b, h, o, w], transposed
     back on host).

The attention phase is ACT/DVE-heavy while the front is PE-heavy, so
front(b1) is interleaved with attn(b0) at group granularity to keep all
engines busy.
"""

import numpy as np

import concourse.bass as bass
import concourse.bacc as bacc
import concourse.tile as tile
from concourse import mybir, masks
from concourse import bass2jax

NCORES = 8
B, C, H, W = 16, 256, 64, 64
O = 256
PIX = H * W
BL = B // NCORES  # batches per core
SCALE = 1.0 / float(np.sqrt(32.0))
EXP_BIAS = -2.0  # softmax-invariant shift keeping exp() well inside fp16 range

FP32 = mybir.dt.float32
FP16 = mybir.dt.float16


def build_kernel(nc: bass.Bass):
    x_in = nc.declare_dram_parameter("x", [BL, C, PIX], FP16, isOutput=False)
    # x with each 64x64 image transposed (w-major pixels); feeds the V
    # projection so V's rows come out w-major.
    xt_in = nc.declare_dram_parameter("xt", [BL, C, PIX], FP16, isOutput=False)
    # host-prepped to the exact SBUF layout [c', proj, cc, o] so the load is
    # one contiguous 3KB descriptor per partition.
    w_in = nc.declare_dram_parameter("w", [128, 3 * 2 * O], FP16, isOutput=False)
    bias_in = nc.declare_dram_parameter("bias", [128, 3 * 2], FP32, isOutput=False)
    # [b, h, o, w]: one contiguous 16KB run per (h, om) -> single-descriptor
    # DMA rows; host transposes back to [b, o, h, w].
    out = nc.declare_dram_parameter("out", [BL, H, O, W], FP16, isOutput=True)

    with tile.TileContext(nc) as tc:
        with (
            tc.tile_pool(name="singles", bufs=1) as singles,
            tc.tile_pool(name="xin", bufs=2) as xpool,
            tc.tile_pool(name="p16", bufs=3) as p16pool,
            tc.tile_pool(name="tsp", bufs=2) as tpool,
            tc.tile_pool(name="obuf", bufs=1) as opool,
            tc.tile_pool(name="small", bufs=6) as small,
            tc.tile_pool(name="psA", bufs=2, space="PSUM") as psA,
            tc.tile_pool(name="psT", bufs=2, space="PSUM") as psT,
            tc.tile_pool(name="psS", bufs=2, space="PSUM") as psS,
            tc.tile_pool(name="psC", bufs=2, space="PSUM") as psC,
        ):
            # ---- constants loaded once; w split per-proj so the first
            # projection only waits on its own third ----
            w_sb = singles.tile([128, 3, 2, O], FP16)  # [c', proj, cc, o]
            w_flat = w_sb.rearrange("p a b c -> p (a b c)")

            def emit_w_third(t):
                nc.sync.dma_start(
                    out=w_flat[:, t * 2 * O : (t + 1) * 2 * O],
                    in_=w_in[:, t * 2 * O : (t + 1) * 2 * O],
                )

            emit_w_third(0)
            bias_sb = singles.tile([128, 3, 2], FP32)  # [o', proj, oc]
            nc.scalar.dma_start(
                out=bias_sb.rearrange("p a b -> p (a b)"), in_=bias_in[:, :]
            )
            expb_sb = singles.tile([128, 1], FP32)
            nc.vector.memset(expb_sb, EXP_BIAS)
            ident = singles.tile([128, 128], FP16)
            masks.make_identity(nc, ident[:])

            tensors = {}
            copy_fns = [
                lambda o_, i_: nc.scalar.copy(o_, i_),
                lambda o_, i_: nc.vector.tensor_copy(o_, i_),
            ]
            # ACT gets 2 of each nt's 3 drain copies (DVE carries the norm
            # multiplies in the attention phase).
            ENG_PAT = [0, 1, 0]

            def emit_transposes(b, tiles, nt):
                t0 = nt * 8
                for ti, (t16, tS) in enumerate(zip(tiles, tensors[b])):
                    pt = psT.tile([128, 8, 128], FP16, tag="pst")
                    for i in range(8):
                        lhsT = t16[:, i, :, :].rearrange("p om w -> p (om w)")
                        nc.tensor.transpose(pt[:, i, :], lhsT, ident)
                    copy_fns[ENG_PAT[ti]](
                        tS[:, :, t0 : t0 + 8], pt.rearrange("p i j -> p j i")
                    )

            def emit_front(b):
                # x cc=0 pieces issue on the SP queue, cc=1 on the ACT queue
                # so the first projection's two halves transfer in parallel.
                # Per-cc tags let batch 1's tiles preload during batch 0's
                # compute. xt (V path, needed a bit later) follows x.
                xq0 = xpool.tile([128, PIX], FP16, tag="xq0")
                xq1 = xpool.tile([128, PIX], FP16, tag="xq1")
                xt0 = xpool.tile([128, PIX], FP16, tag="xtb0")
                xt1 = xpool.tile([128, PIX], FP16, tag="xtb1")
                xsb, xtsb = [xq0, xq1], [xt0, xt1]
                NP = 4

                def emit_x_piece(piece):
                    if piece == 0:
                        # first piece split in half: the opening projection
                        # only waits on a 512-pixel transfer per cc half.
                        for hp in range(2):
                            sl = slice(hp * 512, (hp + 1) * 512)
                            nc.sync.dma_start(
                                out=xq0[:, sl], in_=x_in[b, 0:128, sl]
                            )
                            nc.scalar.dma_start(
                                out=xq1[:, sl], in_=x_in[b, 128:256, sl]
                            )
                    sl = slice(piece * (PIX // NP), (piece + 1) * (PIX // NP))
                    if piece != 0:
                        nc.sync.dma_start(out=xq0[:, sl], in_=x_in[b, 0:128, sl])
                        nc.scalar.dma_start(out=xq1[:, sl], in_=x_in[b, 128:256, sl])
                    nc.sync.dma_start(out=xt0[:, sl], in_=xt_in[b, 0:128, sl])
                    nc.scalar.dma_start(out=xt1[:, sl], in_=xt_in[b, 128:256, sl])

                # only the first pixel range loads up front; later pieces are
                # emitted inside the nt loop so their descriptor-issue cost on
                # the SP/ACT queues doesn't delay the first psum drains. The
                # k/v weight thirds slot in after the first x piece.
                emit_x_piece(0)
                if b == 0:
                    emit_w_third(1)
                    emit_w_third(2)

                # j-major attention layouts: contiguous per-j tiles.
                qS = tpool.tile([128, 128, H], FP16, tag="qS")  # [om*64+w, j, h]
                kS = tpool.tile([128, 128, H], FP16, tag="kS")
                vS = tpool.tile([128, 128, W + 1], FP16, tag="vS")  # [om*64+g, j, w.]
                nc.gpsimd.memset(vS[:, :, W], 1.0)
                tensors[b] = (qS, kS, vS)

                prev = None
                for nt in range(8):
                    if nt in (1, 3, 5):
                        emit_x_piece(nt // 2 + 1)
                    # 8-row staging tiles [j, r(8), om, c]
                    q16 = p16pool.tile([128, 8, 2, W], FP16, tag="q16")
                    k16 = p16pool.tile([128, 8, 2, W], FP16, tag="k16")
                    v16 = p16pool.tile([128, 8, 2, W], FP16, tag="v16")
                    cur = (q16, k16, v16)
                    for proj in range(3):
                        for oc in range(2):
                            ps = psA.tile([128, 512], FP32, tag="ps_proj")
                            xin = xtsb if proj == 2 else xsb
                            for cc in range(2):
                                nc.tensor.matmul(
                                    ps,
                                    lhsT=w_sb[:, proj, cc, oc * 128 : (oc + 1) * 128],
                                    rhs=xin[cc][:, nt * 512 : (nt + 1) * 512],
                                    start=(cc == 0),
                                    stop=(cc == 1),
                                )
                            bias_ap = bias_sb[:, proj, oc : oc + 1]
                            dst = cur[proj][:, :, oc, :]
                            src = ps.rearrange("p (r c) -> p r c", c=W)
                            if oc == 0:
                                nc.scalar.activation(
                                    out=dst,
                                    in_=src,
                                    func=mybir.ActivationFunctionType.Identity,
                                    bias=bias_ap,
                                    scale=1.0,
                                )
                            else:
                                nc.vector.tensor_scalar_add(
                                    out=dst, in0=src, scalar1=bias_ap
                                )
                    if prev is not None:
                        emit_transposes(b, prev[0], prev[1])
                    prev = (cur, nt)
                    yield
                emit_transposes(b, prev[0], prev[1])

            def emit_attn(b):
                qS, kS, vS = tensors[b]
                obuf = opool.tile([128, 128, W], FP16, tag="obuf")  # [om*64+h, j, w]
                JG = 8
                PG = 4
                NJG = 128 // JG

                def emit_S(jg):
                    sp8f = psS.tile([128, 512], FP32, tag="sp8")
                    sp8 = sp8f.rearrange("p (i h) -> p i h", h=H)
                    for i in range(JG):
                        j = jg * JG + i
                        for om in range(2):
                            pr = slice(om * 64, om * 64 + 64)
                            nc.tensor.matmul(
                                sp8[pr, i, :],
                                lhsT=kS[pr, j, :],
                                rhs=qS[pr, j, :],
                                start=True,
                                stop=True,
                            )
                    eS8 = small.tile([128, JG, H], FP16, tag="eS8")
                    nc.scalar.activation(
                        out=eS8,
                        in_=sp8,
                        func=mybir.ActivationFunctionType.Exp,
                        bias=expb_sb,
                        scale=1.0,
                    )
                    return eS8

                def emit_ctx(jg, eS8):
                    for sg in range(2):
                        j0 = jg * JG + sg * PG
                        cp4f = psC.tile([128, 512], FP32, tag="cp4")
                        cp4 = cp4f[:, 0 : PG * (W + 1)].rearrange(
                            "p (i c) -> p i c", c=W + 1
                        )
                        for i in range(PG):
                            j = j0 + i
                            for om in range(2):
                                pr = slice(om * 64, om * 64 + 64)
                                nc.tensor.matmul(
                                    cp4[pr, i, :],
                                    lhsT=eS8[pr, j - jg * JG, :],
                                    rhs=vS[pr, j, :],
                                    start=True,
                                    stop=True,
                                )
                        rz4 = small.tile([128, PG], FP32, tag="rz4")
                        nc.vector.reciprocal(out=rz4, in_=cp4[:, :, W])
                        # bv folded into the V projection bias -> normalize
                        # is one broadcast multiply per group.
                        rzf = rz4[:]
                        rzb = bass.AP(
                            tensor=rzf.tensor,
                            offset=rzf.offset,
                            ap=[rzf.ap[0], rzf.ap[1], [0, W]],
                        )
                        nc.vector.tensor_mul(
                            out=obuf[:, j0 : j0 + PG, :],
                            in0=cp4[:, :, 0:W],
                            in1=rzb,
                        )

                # software pipeline: ctx(jg) runs two S-groups later so the
                # exp() round-trip never stalls the PE. Output stores go out
                # in j-halves so obuf frees for the next batch early.
                pend = []
                done_ctx = 0

                def flush_one():
                    nonlocal done_ctx
                    g, e = pend.pop(0)
                    emit_ctx(g, e)
                    done_ctx += 1
                    # store finished j-ranges as they complete; the final
                    # quarter-stores keep the drain tail short.
                    marks = {
                        32: (0, 32),
                        64: (32, 64),
                        96: (64, 96),
                        120: (96, 120),
                        128: (120, 128),
                    }
                    rng = marks.get(done_ctx * JG)
                    if rng is not None:
                        ja, jb = rng
                        for om in range(2):
                            eng = nc.scalar if (jb == 128 and om == 1) else nc.sync
                            eng.dma_start(
                                out=out[b, :, om * 128 + ja : om * 128 + jb, :],
                                in_=obuf[om * 64 : om * 64 + 64, ja:jb, :],
                            )

                for jg in range(NJG):
                    pend.append((jg, emit_S(jg)))
                    if len(pend) == 2:
                        flush_one()
                    yield
                while pend:
                    flush_one()

            def drain(gen):
                for _ in gen:
                    pass

            # F0 | F1 interleaved with A0 | A1
            drain(emit_front(0))
            f1, a0 = emit_front(1), emit_attn(0)
            while True:
                done_a = next(a0, "end") == "end"
                done_a = (next(a0, "end") == "end") or done_a
                done_f = next(f1, "end") == "end"
                if done_f and done_a:
                    break
            drain(emit_attn(1))
    return nc


_NC_CACHE = {}


def get_nc():
    if "nc" not in _NC_CACHE:
        nc = bacc.Bacc(None, target_bir_lowering=False)
        build_kernel(nc)
        nc.finalize()
        _NC_CACHE["nc"] = nc
    return _NC_CACHE["nc"]


def prep_in_maps(x, wq, bq, wk, bk, wv, bv):
    wT = np.stack(
        [
            np.ascontiguousarray((wq * SCALE).T),
            np.ascontiguousarray(wk.T),
            np.ascontiguousarray(wv.T),
        ]
    ).astype(np.float16)
    # device layout [c', proj, cc, o] flattened: w_sb[c,t,cc,o] = wT[t, cc*128+c, o]
    w_dev = np.ascontiguousarray(
        wT.reshape(3, 2, 128, O).transpose(2, 0, 1, 3).reshape(128, 3 * 2 * O)
    )
    # bv is folded into the V projection bias: softmax weights sum to 1, so
    # (sum_g P*(V+bv)) == (sum_g P*V) + bv exactly.
    biases = np.stack([bq * SCALE, bk, bv]).astype(np.float32)
    bias_dev = np.ascontiguousarray(
        biases.reshape(3, 2, 128).transpose(2, 0, 1).reshape(128, 6)
    )
    x16 = x.astype(np.float16)
    xs = np.ascontiguousarray(x16.reshape(NCORES, BL, C, PIX))
    xts = np.ascontiguousarray(
        x16.reshape(NCORES, BL, C, H, W).transpose(0, 1, 2, 4, 3)
    ).reshape(NCORES, BL, C, PIX)
    return [
        {"x": xs[i], "xt": xts[i], "w": w_dev, "bias": bias_dev}
        for i in range(NCORES)
    ]


def gather_outs(results):
    # device out is [BL, H, O, W]; transpose back to [BL, O, H, W]
    outs = [
        np.asarray(r["out"]).reshape(BL, H, O, W).transpose(0, 2, 1, 3)
        for r in results
    ]
    return np.concatenate(outs, axis=0).astype(np.float32)


def kernel(x, wq, bq, wk, bk, wv, bv):
    nc = get_nc()
    in_maps = prep_in_maps(x, wq, bq, wk, bk, wv, bv)
    results = bass2jax.run_bass_via_pjrt(nc, in_maps, n_cores=NCORES)
    return gather_outs(results)


# revision 70
# speedup vs baseline: 1.0122x; 1.0122x over previous
"""Trainium2 Bass kernel for nn_MultiHeadedSelfAttention_5179730559275.

Reference math (per batch b):
  q = wq @ x + bq ; k = wk @ x + bk ; v = wv @ x + bv        (1x1 conv, C=256 -> O=256)
  per o-channel (o = head*32 + d), with Q_o,K_o,V_o = 64x64 images [H,W]:
    S_o = Q_o @ K_o^T / sqrt(32); P_o = softmax(S_o, axis=-1); ctx_o = P_o @ V_o

Sharding: data-parallel over batch, 2 batches per core on 8 cores.

Per-core pipeline (per batch):
  1. fp16 projections on PE -> psum [o', 512]; ACT/DVE copies add bias +
     cast fp16 into pair-interleaved 8-row tiles [j, r, om, c] (j = o mod
     128, om pairs o with o+128). V projects from host-transposed x so its
     rows are w-major.
  2. PE transposes (is_transpose matmul vs fp16 identity) flip 128x128
     slices [j, (om,c)] -> [(om,c), j], 8 per psum bank; one batched copy
     per bank builds j-major layouts (contiguous per-j tiles so PE weight
     loads / moving fetches run at 1 elem/cycle):
       qS/kS: [om*64 + w, j, h] ; vS: [om*64 + g, j, w] (+ ones col for Z)
     Transposes for row-group nt interleave with the projections of nt+1
     so the PE never waits on the psT drain copies.
  3. Attention per pair j: quadrant matmuls (K=64 at partition bases 0/64):
       S^T psum [om*64+g, h] ; exp (ACT, bias -2) -> eS fp16
       ctx psum [om*64+h, 0:64]=E^T.T@V, col 64 = Z (ones column)
     normalize (bv folded into V bias): obuf = psum * (1/Z); one 16KB-
     descriptor DMA store per om per batch (DRAM [b, h, o, w], transposed
     back on host).

The attention phase is ACT/DVE-heavy while the front is PE-heavy, so
front(b1) is interleaved with attn(b0) at group granularity to keep all
engines busy.
"""

import numpy as np

import concourse.bass as bass
import concourse.bacc as bacc
import concourse.tile as tile
from concourse import mybir, masks
from concourse import bass2jax

NCORES = 8
B, C, H, W = 16, 256, 64, 64
O = 256
PIX = H * W
BL = B // NCORES  # batches per core
SCALE = 1.0 / float(np.sqrt(32.0))
EXP_BIAS = -2.0  # softmax-invariant shift keeping exp() well inside fp16 range

FP32 = mybir.dt.float32
FP16 = mybir.dt.float16


def build_kernel(nc: bass.Bass):
    x_in = nc.declare_dram_parameter("x", [BL, C, PIX], FP16, isOutput=False)
    # x with each 64x64 image transposed (w-major pixels); feeds the V
    # projection so V's rows come out w-major.
    xt_in = nc.declare_dram_parameter("xt", [BL, C, PIX], FP16, isOutput=False)
    # host-prepped to the exact SBUF layout [c', proj, cc, o] so the load is
    # one contiguous 3KB descriptor per partition.
    w_in = nc.declare_dram_parameter("w", [128, 3 * 2 * O], FP16, isOutput=False)
    bias_in = nc.declare_dram_parameter("bias", [128, 3 * 2], FP32, isOutput=False)
    # [b, h, o, w]: one contiguous 16KB run per (h, om) -> single-descriptor
    # DMA rows; host transposes back to [b, o, h, w].
    out = nc.declare_dram_parameter("out", [BL, H, O, W], FP16, isOutput=True)

    with tile.TileContext(nc) as tc:
        with (
            tc.tile_pool(name="singles", bufs=1) as singles,
            tc.tile_pool(name="xin", bufs=2) as xpool,
            tc.tile_pool(name="p16", bufs=3) as p16pool,
            tc.tile_pool(name="tsp", bufs=2) as tpool,
            tc.tile_pool(name="obuf", bufs=1) as opool,
            tc.tile_pool(name="small", bufs=6) as small,
            tc.tile_pool(name="psA", bufs=2, space="PSUM") as psA,
            tc.tile_pool(name="psT", bufs=2, space="PSUM") as psT,
            tc.tile_pool(name="psS", bufs=2, space="PSUM") as psS,
            tc.tile_pool(name="psC", bufs=2, space="PSUM") as psC,
        ):
            # ---- constants loaded once; w split per-proj so the first
            # projection only waits on its own third ----
            w_sb = singles.tile([128, 3, 2, O], FP16)  # [c', proj, cc, o]
            w_flat = w_sb.rearrange("p a b c -> p (a b c)")

            def emit_w_third(t):
                nc.sync.dma_start(
                    out=w_flat[:, t * 2 * O : (t + 1) * 2 * O],
                    in_=w_in[:, t * 2 * O : (t + 1) * 2 * O],
                )

            emit_w_third(0)
            bias_sb = singles.tile([128, 3, 2], FP32)  # [o', proj, oc]
            nc.scalar.dma_start(
                out=bias_sb.rearrange("p a b -> p (a b)"), in_=bias_in[:, :]
            )
            expb_sb = singles.tile([128, 1], FP32)
            nc.vector.memset(expb_sb, EXP_BIAS)
            ident = singles.tile([128, 128], FP16)
            masks.make_identity(nc, ident[:])

            tensors = {}
            copy_fns = [
                lambda o_, i_: nc.scalar.copy(o_, i_),
                lambda o_, i_: nc.vector.tensor_copy(o_, i_),
            ]
            # ACT gets 2 of each nt's 3 drain copies (DVE carries the norm
            # multiplies in the attention phase).
            ENG_PAT = [0, 1, 0]

            def emit_transposes(b, tiles, nt):
                t0 = nt * 8
                for ti, (t16, tS) in enumerate(zip(tiles, tensors[b])):
                    pt = psT.tile([128, 8, 128], FP16, tag="pst")
                    for i in range(8):
                        lhsT = t16[:, i, :, :].rearrange("p om w -> p (om w)")
                        nc.tensor.transpose(pt[:, i, :], lhsT, ident)
                    copy_fns[ENG_PAT[ti]](
                        tS[:, :, t0 : t0 + 8], pt.rearrange("p i j -> p j i")
                    )

            def emit_front(b):
                # x cc=0 pieces issue on the SP queue, cc=1 on the ACT queue
                # so the first projection's two halves transfer in parallel.
                # Per-cc tags let batch 1's tiles preload during batch 0's
                # compute. xt (V path, needed a bit later) follows x.
                xq0 = xpool.tile([128, PIX], FP16, tag="xq0")
                xq1 = xpool.tile([128, PIX], FP16, tag="xq1")
                xt0 = xpool.tile([128, PIX], FP16, tag="xtb0")
                xt1 = xpool.tile([128, PIX], FP16, tag="xtb1")
                xsb, xtsb = [xq0, xq1], [xt0, xt1]
                NP = 4

                def emit_x_piece(piece):
                    sl = slice(piece * (PIX // NP), (piece + 1) * (PIX // NP))
                    nc.sync.dma_start(out=xq0[:, sl], in_=x_in[b, 0:128, sl])
                    nc.scalar.dma_start(out=xq1[:, sl], in_=x_in[b, 128:256, sl])
                    nc.sync.dma_start(out=xt0[:, sl], in_=xt_in[b, 0:128, sl])
                    nc.scalar.dma_start(out=xt1[:, sl], in_=xt_in[b, 128:256, sl])

                # only the first pixel range loads up front; later pieces are
                # emitted inside the nt loop so their descriptor-issue cost on
                # the SP/ACT queues doesn't delay the first psum drains. The
                # k/v weight thirds slot in after the first x piece.
                emit_x_piece(0)
                if b == 0:
                    emit_w_third(1)
                    emit_w_third(2)

                # j-major attention layouts: contiguous per-j tiles.
                qS = tpool.tile([128, 128, H], FP16, tag="qS")  # [om*64+w, j, h]
                kS = tpool.tile([128, 128, H], FP16, tag="kS")
                vS = tpool.tile([128, 128, W + 1], FP16, tag="vS")  # [om*64+g, j, w.]
                nc.gpsimd.memset(vS[:, :, W], 1.0)
                tensors[b] = (qS, kS, vS)

                prev = None
                for nt in range(8):
                    if nt in (1, 3, 5):
                        emit_x_piece(nt // 2 + 1)
                    # 8-row staging tiles [j, r(8), om, c]
                    q16 = p16pool.tile([128, 8, 2, W], FP16, tag="q16")
                    k16 = p16pool.tile([128, 8, 2, W], FP16, tag="k16")
                    v16 = p16pool.tile([128, 8, 2, W], FP16, tag="v16")
                    cur = (q16, k16, v16)
                    for proj in range(3):
                        for oc in range(2):
                            ps = psA.tile([128, 512], FP32, tag="ps_proj")
                            xin = xtsb if proj == 2 else xsb
                            for cc in range(2):
                                nc.tensor.matmul(
                                    ps,
                                    lhsT=w_sb[:, proj, cc, oc * 128 : (oc + 1) * 128],
                                    rhs=xin[cc][:, nt * 512 : (nt + 1) * 512],
                                    start=(cc == 0),
                                    stop=(cc == 1),
                                )
                            bias_ap = bias_sb[:, proj, oc : oc + 1]
                            dst = cur[proj][:, :, oc, :]
                            src = ps.rearrange("p (r c) -> p r c", c=W)
                            if oc == 0:
                                nc.scalar.activation(
                                    out=dst,
                                    in_=src,
                                    func=mybir.ActivationFunctionType.Identity,
                                    bias=bias_ap,
                                    scale=1.0,
                                )
                            else:
                                nc.vector.tensor_scalar_add(
                                    out=dst, in0=src, scalar1=bias_ap
                                )
                    if prev is not None:
                        emit_transposes(b, prev[0], prev[1])
                    prev = (cur, nt)
                    yield
                emit_transposes(b, prev[0], prev[1])

            def emit_attn(b):
                qS, kS, vS = tensors[b]
                obuf = opool.tile([128, 128, W], FP16, tag="obuf")  # [om*64+h, j, w]
                JG = 8
                PG = 4
                NJG = 128 // JG

                def emit_S(jg):
                    sp8f = psS.tile([128, 512], FP32, tag="sp8")
                    sp8 = sp8f.rearrange("p (i h) -> p i h", h=H)
                    for i in range(JG):
                        j = jg * JG + i
                        for om in range(2):
                            pr = slice(om * 64, om * 64 + 64)
                            nc.tensor.matmul(
                                sp8[pr, i, :],
                                lhsT=kS[pr, j, :],
                                rhs=qS[pr, j, :],
                                start=True,
                                stop=True,
                            )
                    eS8 = small.tile([128, JG, H], FP16, tag="eS8")
                    nc.scalar.activation(
                        out=eS8,
                        in_=sp8,
                        func=mybir.ActivationFunctionType.Exp,
                        bias=expb_sb,
                        scale=1.0,
                    )
                    return eS8

                def emit_ctx(jg, eS8):
                    for sg in range(2):
                        j0 = jg * JG + sg * PG
                        cp4f = psC.tile([128, 512], FP32, tag="cp4")
                        cp4 = cp4f[:, 0 : PG * (W + 1)].rearrange(
                            "p (i c) -> p i c", c=W + 1
                        )
                        for i in range(PG):
                            j = j0 + i
                            for om in range(2):
                                pr = slice(om * 64, om * 64 + 64)
                                nc.tensor.matmul(
                                    cp4[pr, i, :],
                                    lhsT=eS8[pr, j - jg * JG, :],
                                    rhs=vS[pr, j, :],
                                    start=True,
                                    stop=True,
                                )
                        rz4 = small.tile([128, PG], FP32, tag="rz4")
                        nc.vector.reciprocal(out=rz4, in_=cp4[:, :, W])
                        # bv folded into the V projection bias -> normalize
                        # is one broadcast multiply per group.
                        rzf = rz4[:]
                        rzb = bass.AP(
                            tensor=rzf.tensor,
                            offset=rzf.offset,
                            ap=[rzf.ap[0], rzf.ap[1], [0, W]],
                        )
                        nc.vector.tensor_mul(
                            out=obuf[:, j0 : j0 + PG, :],
                            in0=cp4[:, :, 0:W],
                            in1=rzb,
                        )

                # software pipeline: ctx(jg) runs two S-groups later so the
                # exp() round-trip never stalls the PE. Output stores go out
                # in j-halves so obuf frees for the next batch early.
                pend = []
                done_ctx = 0

                def flush_one():
                    nonlocal done_ctx
                    g, e = pend.pop(0)
                    emit_ctx(g, e)
                    done_ctx += 1
                    # store finished j-ranges as they complete; the final
                    # quarter-stores keep the drain tail short.
                    marks = {
                        32: (0, 32),
                        64: (32, 64),
                        96: (64, 96),
                        120: (96, 120),
                        128: (120, 128),
                    }
                    rng = marks.get(done_ctx * JG)
                    if rng is not None:
                        ja, jb = rng
                        for om in range(2):
                            nc.sync.dma_start(
                                out=out[b, :, om * 128 + ja : om * 128 + jb, :],
                                in_=obuf[om * 64 : om * 64 + 64, ja:jb, :],
                            )

                for jg in range(NJG):
                    pend.append((jg, emit_S(jg)))
                    if len(pend) == 2:
                        flush_one()
                    yield
                while pend:
                    flush_one()

            def drain(gen):
                for _ in gen:
                    pass

            # F0 | F1 interleaved with A0 | A1
            drain(emit_front(0))
            f1, a0 = emit_front(1), emit_attn(0)
            while True:
                done_a = next(a0, "end") == "end"
                done_a = (next(a0, "end") == "end") or done_a
                done_f = next(f1, "end") == "end"
                if done_f and done_a:
                    break
            drain(emit_attn(1))
    return nc


_NC_CACHE = {}


def get_nc():
    if "nc" not in _NC_CACHE:
        nc = bacc.Bacc(None, target_bir_lowering=False)
        build_kernel(nc)
        nc.finalize()
        _NC_CACHE["nc"] = nc
    return _NC_CACHE["nc"]


def prep_in_maps(x, wq, bq, wk, bk, wv, bv):
    wT = np.stack(
        [
            np.ascontiguousarray((wq * SCALE).T),
            np.ascontiguousarray(wk.T),
            np.ascontiguousarray(wv.T),
        ]
    ).astype(np.float16)
    # device layout [c', proj, cc, o] flattened: w_sb[c,t,cc,o] = wT[t, cc*128+c, o]
    w_dev = np.ascontiguousarray(
        wT.reshape(3, 2, 128, O).transpose(2, 0, 1, 3).reshape(128, 3 * 2 * O)
    )
    # bv is folded into the V projection bias: softmax weights sum to 1, so
    # (sum_g P*(V+bv)) == (sum_g P*V) + bv exactly.
    biases = np.stack([bq * SCALE, bk, bv]).astype(np.float32)
    bias_dev = np.ascontiguousarray(
        biases.reshape(3, 2, 128).transpose(2, 0, 1).reshape(128, 6)
    )
    x16 = x.astype(np.float16)
    xs = np.ascontiguousarray(x16.reshape(NCORES, BL, C, PIX))
    xts = np.ascontiguousarray(
        x16.reshape(NCORES, BL, C, H, W).transpose(0, 1, 2, 4, 3)
    ).reshape(NCORES, BL, C, PIX)
    return [
        {"x": xs[i], "xt": xts[i], "w": w_dev, "bias": bias_dev}
        for i in range(NCORES)
    ]


def gather_outs(results):
    # device out is [BL, H, O, W]; transpose back to [BL, O, H, W]
    outs = [
        np.asarray(r["out"]).reshape(BL, H, O, W).transpose(0, 2, 1, 3)
        for r in results
    ]
    return np.concatenate(outs, axis=0).astype(np.float32)


def kernel(x, wq, bq, wk, bk, wv, bv):
    nc = get_nc()
    in_maps = prep_in_maps(x, wq, bq, wk, bk, wv, bv)
    results = bass2jax.run_bass_via_pjrt(nc, in_maps, n_cores=NCORES)
    return gather_outs(results)
